# revision 15
# baseline (speedup 1.0000x reference)
"""GQA attention (B=2, S=2048, DIM=2048, H=16, KVH=4, HD=128, RoPE, causal)
on 8 TRN2 NeuronCores.

Sharding: core c -> batch b = c//4, head-group g = c%4 (q heads 4g..4g+3,
which map exactly to kv head g). Each core computes the partial output
attn_heads @ wo_slice.T  ([S, DIM]); the host sums the 4 partials per batch.

Device layout (everything "transposed", feature-major):
  xT   [DIM, S]   bf16   x[b].T
  wqT  [DIM, 512] bf16   (per-head even/odd-permuted, 1/sqrt(HD)-scaled) wq.T
  wkT  [DIM, 128] bf16   permuted wk.T
  wvT  [DIM, 128] bf16   wv.T (not permuted; v is not roped)
  woT  [512, DIM] bf16   wo[:, cols].T
  cosT [128, S]   bf16   [cos; cos] rope table, frequency-major, duplicated
  sinT [128, S]   bf16   [-sin; sin] sign-folded rope table

The per-head even/odd permutation (rows [0,2,..,126,1,3,..,127]) turns RoPE
pair-interleaving into contiguous half-partitions; q.k dot products are
invariant because q and k are permuted identically.

Attention is computed in transposed score layout: scoresT[k, q] so that
probsT feeds the PV matmul directly (lhsT = v natural layout) and attnT
falls out in [hd, q] = exactly the lhsT the output projection needs.

Schedule notes:
  - softmax denominators: DVE accumulates the (masked) probs tiles, then
    ONE ones-matmul per (head, chunk) gives the partition-replicated sum
    (instead of a ones-matmul per k-tile re-streaming all probs).
  - input DMA issue on the Sync queue costs ~1.6us per descriptor set and
    is the phase-A bottleneck, so xT is loaded as 8x 1MB pair-tiles
    (halving the issue count) with a depth-2 completion chain; K-proj and
    Q-head0 run dt-outer consuming tiles as they land, with K four steps
    ahead of Q (wq0 arrives after xt0). Dummy matmuls ramp the PE clock
    governor (HAM) during the DMA lead-in.
  - PSUM: 5-buf rotating pool for scores/projections (pipeline elasticity)
    + double-buffered attention accumulator + softmax-sum bank.
  - outputs are merged to [128, 2048] tiles (4KB DRAM rows), DMA'd in two
    halves so transfers start before the whole block is copied.
"""

import math
import sys

import numpy as np

try:
    import concourse.bacc as bacc  # noqa: F401
except ImportError:
    sys.path.insert(0, "/opt/trn_rl_repo")

import ml_dtypes
import concourse.bacc as bacc
import concourse.tile as tile
from concourse import mybir
from concourse.bass_utils import run_bass_kernel_spmd
from concourse.bass import _add_dep_helper

BF16 = mybir.dt.bfloat16
F32 = mybir.dt.float32

B, S, DIM = 2, 2048, 2048
H, KVH, HD = 16, 4, 128
N_CORES = 8
P = 128
D_T = DIM // P      # 16 contraction tiles
NH = H // KVH       # 4 q-heads per core
QC = 512            # q-chunk (matmul moving free dim)
QB = S // QC        # 4 q-chunks
S_T = S // P        # 16 s-tiles / k-tiles
N_WARM = 7          # dummy warm-up matmuls to ramp HAM

_cached = {}


def _build_nc():
    nc = bacc.Bacc("TRN2", target_bir_lowering=False, debug=False,
                   num_devices=N_CORES)
    xT = nc.dram_tensor("xT", [DIM, S], BF16, kind="ExternalInput").ap()
    wqT = nc.dram_tensor("wqT", [DIM, NH * HD], BF16, kind="ExternalInput").ap()
    wkT = nc.dram_tensor("wkT", [DIM, HD], BF16, kind="ExternalInput").ap()
    wvT = nc.dram_tensor("wvT", [DIM, HD], BF16, kind="ExternalInput").ap()
    woT = nc.dram_tensor("woT", [NH * HD, DIM], BF16, kind="ExternalInput").ap()
    cosT = nc.dram_tensor("cosT", [HD, S], BF16, kind="ExternalInput").ap()
    sinT = nc.dram_tensor("sinT", [HD, S], BF16, kind="ExternalInput").ap()
    out = nc.dram_tensor("out", [S, DIM], BF16, kind="ExternalOutput").ap()

    with tile.TileContext(nc) as tc:
        _build_kernel(tc, xT, wqT, wkT, wvT, woT, cosT, sinT, out)
    nc.compile()
    return nc


def _build_kernel(tc, xT, wqT, wkT, wvT, woT, cosT, sinT, out):
    nc = tc.nc
    Exp = mybir.ActivationFunctionType.Exp

    with (
        tc.tile_pool(name="const", bufs=1) as const,
        tc.tile_pool(name="big", bufs=1) as big,
        tc.tile_pool(name="rtmp", bufs=8) as rtmp,
        tc.tile_pool(name="probs", bufs=9) as probs_pool,
        tc.tile_pool(name="pracc", bufs=3) as pracc_pool,
        tc.tile_pool(name="attn", bufs=6) as attn_pool,
        tc.tile_pool(name="rz", bufs=3) as rz_pool,
        tc.tile_pool(name="osb", bufs=2) as osb_pool,
        tc.tile_pool(name="ps", bufs=5, space="PSUM") as ps_pool,
        tc.tile_pool(name="ps_at", bufs=2, space="PSUM") as ps_at_pool,
        tc.tile_pool(name="ps_z", bufs=1, space="PSUM") as ps_z_pool,
    ):
        # ---- constants + HAM warm-up ----
        ones = const.tile([P, P], BF16, name="ones")
        nc.vector.memset(ones, 1.0)
        dum = const.tile([P, QC], BF16, name="dum")
        nc.vector.memset(dum, 0.25)
        warm_ps = ps_at_pool.tile([P, QC], F32, name="at")
        for _ in range(N_WARM):
            nc.tensor.matmul(warm_ps[:, 0:384], lhsT=dum[:, 0:P],
                             rhs=dum[:, 0:384], start=True, stop=True)

        cos_sb = const.tile([HD, S], BF16, name="cos")
        sin_sb = const.tile([HD, S], BF16, name="sin")

        # ---- input DMAs ----
        # xT as 8 pair-tiles of 1MB: half the Sync-issue cost per byte.
        # Depth-2 chain: ~2 transfers in flight -> near-aggregate bandwidth
        # with in-order arrival. Deferred weights chain behind xT.
        wk_sb = big.tile([P, D_T, HD], BF16, name="wk")
        nc.sync.dma_start(out=wk_sb, in_=wkT.rearrange("(t p) j -> p t j", p=P))

        xt_tiles = {}
        xp_dmas = []
        # dt 0 and 1 as four 0.25MB transfers (fast first arrival: K-proj
        # starts on xt0's first half), dt 2..15 as 1MB pair-tiles (half the
        # Sync-issue cost per byte), depth-2 completion chain throughout.
        t01 = big.tile([P, 2, S], BF16, name="xp0")
        first_dmas = []
        for half in range(4):
            dma = nc.sync.dma_start(
                out=t01[:, half // 2, (half % 2) * S // 2:
                        (half % 2 + 1) * S // 2],
                in_=xT[(half // 2) * P:(half // 2 + 1) * P,
                       (half % 2) * S // 2:(half % 2 + 1) * S // 2])
            if half >= 2:
                _add_dep_helper(dma.ins, first_dmas[half - 2].ins, sync=True,
                                reason="stagger first xT quarters")
            first_dmas.append(dma)
        for half in range(2):
            for sc in range(QB):
                xt_tiles[(half, sc)] = t01[:, half, sc * QC:(sc + 1) * QC]
        xp_dmas.append(first_dmas[3])
        for dp in range(1, D_T // 2):
            t = big.tile([P, 2, S], BF16, name=f"xp{dp}")
            dma = nc.sync.dma_start(
                out=t,
                in_=xT[dp * 2 * P:(dp + 1) * 2 * P, :].rearrange(
                    "(t p) s -> p t s", p=P))
            if dp == 1:
                _add_dep_helper(dma.ins, first_dmas[1].ins, sync=True,
                                reason="stagger xT pair load")
            else:
                _add_dep_helper(dma.ins, xp_dmas[dp - 2].ins, sync=True,
                                reason="stagger xT pair load")
            for half in range(2):
                for sc in range(QB):
                    xt_tiles[(2 * dp + half, sc)] = \
                        t[:, half, sc * QC:(sc + 1) * QC]
            xp_dmas.append(dma)

        wq_sb = big.tile([P, D_T, NH * HD], BF16, name="wq")
        wq_r = wqT.rearrange("(t p) j -> p t j", p=P)
        # head 0 is needed early (runs interleaved with K-proj)
        dma_wq0 = nc.sync.dma_start(out=wq_sb[:, :, 0:HD],
                                    in_=wq_r[:, :, 0:HD])
        _add_dep_helper(dma_wq0.ins, xp_dmas[0].ins, sync=True,
                        reason="wq0 right after first xT pair")
        for _src, _dst in ((cosT, cos_sb), (sinT, sin_sb)):
            _d = nc.sync.dma_start(out=_dst, in_=_src)
            _add_dep_helper(_d.ins, xp_dmas[6].ins, sync=True,
                            reason="rope tables near end of xT")
        dma_wq123 = nc.sync.dma_start(out=wq_sb[:, :, HD:NH * HD],
                                      in_=wq_r[:, :, HD:NH * HD])
        _add_dep_helper(dma_wq123.ins, xp_dmas[7].ins, sync=True,
                        reason="wq heads 1-3 after xT")
        wv_sb = big.tile([P, D_T, HD], BF16, name="wv")
        dma_wv = nc.sync.dma_start(out=wv_sb,
                                   in_=wvT.rearrange("(t p) j -> p t j", p=P))
        _add_dep_helper(dma_wv.ins, dma_wq123.ins, sync=True,
                        reason="wv after wq123")
        wo_sb = big.tile([P, NH, DIM], BF16, name="wo")
        dma_wo = nc.sync.dma_start(out=wo_sb,
                                   in_=woT.rearrange("(t p) d -> p t d", p=P))
        _add_dep_helper(dma_wo.ins, dma_wv.ins, sync=True, reason="wo last")

        qT = big.tile([P, NH, S], BF16, name="qT")
        kT = big.tile([P, S], BF16, name="kT")
        v_sb = big.tile([P, S_T, HD], BF16, name="v")

        def rope(dst, ps, sc):
            """dst (bf16 [128,512] slice) <- rotate(ps).

            ACT stages ps to bf16 SBUF twice (straight + halves swapped via
            ScalarE partition-shifting copies); DVE then runs three
            full-width ops against the sign-folded tables:
            dst = st*[cos;cos] + sw*[-sin;sin]."""
            h = HD // 2
            st = rtmp.tile([P, QC], BF16, name="rst")
            sw = rtmp.tile([P, QC], BF16, name="rsw")
            nc.scalar.copy(out=st, in_=ps)
            nc.scalar.copy(out=sw[0:h, :], in_=ps[h:P, :])
            nc.scalar.copy(out=sw[h:P, :], in_=ps[0:h, :])
            cos_c = cos_sb[:, sc * QC:(sc + 1) * QC]
            sin_c = sin_sb[:, sc * QC:(sc + 1) * QC]
            t0 = rtmp.tile([P, QC], BF16, name="rt")
            t1 = rtmp.tile([P, QC], BF16, name="rt")
            nc.vector.tensor_mul(t0, st, cos_c)
            nc.vector.tensor_mul(t1, sw, sin_c)
            nc.vector.tensor_add(dst, t0, t1)

        # ---- K projection + Q head-0, dt-outer ----
        # K runs 4 dt-tiles ahead of Q-h0 so the PE starts as soon as the
        # first xT pair lands (wq0 arrives a bit later).
        kps = [ps_pool.tile([P, QC], F32, name="ps") for _ in range(QB)]
        q0ps = [ps_pool.tile([P, QC], F32, name="ps"),
                ps_at_pool.tile([P, QC], F32, name="at"),
                ps_at_pool.tile([P, QC], F32, name="at"),
                ps_z_pool.tile([P, QC], F32, name="z")]

        def kmm(dt):
            for sc in range(QB):
                nc.tensor.matmul(kps[sc], lhsT=wk_sb[:, dt, :],
                                 rhs=xt_tiles[(dt, sc)],
                                 start=(dt == 0), stop=(dt == D_T - 1))

        def q0mm(dt):
            for sc in range(QB):
                nc.tensor.matmul(q0ps[sc], lhsT=wq_sb[:, dt, 0:HD],
                                 rhs=xt_tiles[(dt, sc)],
                                 start=(dt == 0), stop=(dt == D_T - 1))

        for dt in range(4):
            kmm(dt)
        for dt in range(4, D_T):
            kmm(dt)
            q0mm(dt - 4)
        for dt in range(D_T - 4, D_T):
            q0mm(dt)

        # K ropes first: K's psums (the ps bufs Q-head1 needs) finished 4
        # dt-steps before q0's, so they drain while q0's last matmuls run.
        for sc in range(QB):
            rope(kT[:, sc * QC:(sc + 1) * QC], kps[sc], sc)
        for sc in range(QB):
            rope(qT[:, 0, sc * QC:(sc + 1) * QC], q0ps[sc], sc)

        # ---- Q heads 1..3, dt-outer per head ----
        for hh in range(1, NH):
            qps = [ps_pool.tile([P, QC], F32, name="ps") for _ in range(QB)]
            for dt in range(D_T):
                for sc in range(QB):
                    nc.tensor.matmul(
                        qps[sc], lhsT=wq_sb[:, dt, hh * HD:(hh + 1) * HD],
                        rhs=xt_tiles[(dt, sc)],
                        start=(dt == 0), stop=(dt == D_T - 1))
            for sc in range(QB):
                rope(qT[:, hh, sc * QC:(sc + 1) * QC], qps[sc], sc)

        # ---- V projection (natural [s, hd] layout) ----
        for st in range(S_T):
            ps = ps_pool.tile([P, QC], F32, name="ps")
            for dt in range(D_T):
                nc.tensor.matmul(
                    ps[:, 0:HD],
                    lhsT=xt_tiles[(dt, st // 4)][:, (st % 4) * P:(st % 4 + 1) * P],
                    rhs=wv_sb[:, dt, :],
                    start=(dt == 0), stop=(dt == D_T - 1))
            nc.scalar.copy(out=v_sb[:, st, :], in_=ps[:, 0:HD])

        # ---- attention + output projection, per q-chunk ----
        # Chunks run longest-first so the serial tail is the shortest chunk.
        chunks = [(1536, 512), (1024, 512), (512, 512), (0, 512)]
        for ci, (q0, qw) in enumerate(chunks):
            nk = (q0 + qw) // P  # causal k-tiles for this q-chunk
            attn_tiles = []
            for hh in range(NH):
                at_ps = ps_at_pool.tile([P, qw], F32, name="at")
                pr_acc = pracc_pool.tile([P, qw], BF16, name="pracc")
                for k in range(nk):
                    # On diagonal tiles only columns q0+off.. are causally
                    # valid; narrow every stage to that width.
                    off = max(0, k * P - q0)
                    w = qw - off
                    diag = k * P >= q0
                    sc_ps = ps_pool.tile([P, QC], F32, name="ps")
                    nc.tensor.matmul(sc_ps[:, 0:w], lhsT=kT[:, k * P:(k + 1) * P],
                                     rhs=qT[:, hh, q0 + off:q0 + qw],
                                     start=True, stop=True)
                    pr = probs_pool.tile([P, QC], BF16, name="pr")
                    nc.scalar.activation(out=pr[:, 0:w], in_=sc_ps[:, 0:w],
                                         func=Exp)
                    if diag:  # zero where c' < r
                        nc.gpsimd.affine_select(
                            out=pr[:, 0:w], in_=pr[:, 0:w],
                            compare_op=mybir.AluOpType.is_ge,
                            fill=0.0, base=0, pattern=[[1, w]],
                            channel_multiplier=-1)
                    nc.tensor.matmul(at_ps[:, off:qw], lhsT=v_sb[:, k, :],
                                     rhs=pr[:, 0:w],
                                     start=(k == 0), stop=(k == nk - 1))
                    if k == 0:
                        nc.vector.tensor_copy(out=pr_acc, in_=pr[:, 0:qw])
                    else:
                        nc.vector.tensor_add(pr_acc[:, off:qw],
                                             pr_acc[:, off:qw], pr[:, 0:w])
                z_ps = ps_z_pool.tile([P, qw], F32, name="z")
                nc.tensor.matmul(z_ps, lhsT=ones, rhs=pr_acc,
                                 start=True, stop=True)
                rz = rz_pool.tile([P, qw], F32, name="rz")
                nc.vector.reciprocal_approx_fast(out=rz, in_=z_ps)
                a_sb = attn_pool.tile([P, qw], BF16, name="attn")
                nc.vector.tensor_mul(a_sb, at_ps, rz)
                attn_tiles.append(a_sb)

            for st in range(qw // P):
                row0 = q0 + st * P
                o_sb = osb_pool.tile([P, DIM], BF16, name="osb")
                for dc in range(DIM // QC):
                    op_ps = ps_pool.tile([P, QC], F32, name="ps")
                    for j in range(NH):
                        nc.tensor.matmul(
                            op_ps, lhsT=attn_tiles[j][:, st * P:(st + 1) * P],
                            rhs=wo_sb[:, j, dc * QC:(dc + 1) * QC],
                            start=(j == 0), stop=(j == NH - 1))
                    if (st * 4 + dc) % 2 == 0:
                        nc.scalar.copy(out=o_sb[:, dc * QC:(dc + 1) * QC],
                                       in_=op_ps)
                    else:
                        nc.vector.tensor_copy(out=o_sb[:, dc * QC:(dc + 1) * QC],
                                              in_=op_ps)
                    if dc == 1:
                        nc.sync.dma_start(out=out[row0:row0 + P, 0:2 * QC],
                                          in_=o_sb[:, 0:2 * QC])
                nc.sync.dma_start(out=out[row0:row0 + P, 2 * QC:DIM],
                                  in_=o_sb[:, 2 * QC:DIM])


def _get_nc():
    if "nc" not in _cached:
        _cached["nc"] = _build_nc()
    return _cached["nc"]


def _prep_in_maps(x, freqs_cis, wq, wk, wv, wo):
    bf = ml_dtypes.bfloat16
    perm = np.concatenate([np.arange(0, HD, 2), np.arange(1, HD, 2)])
    scale = 1.0 / math.sqrt(HD)
    wq_p = (wq.reshape(H, HD, DIM)[:, perm, :] * scale).astype(np.float32)
    wk_p = wk.reshape(KVH, HD, DIM)[:, perm, :]
    cos_h = np.ascontiguousarray(freqs_cis[:, :, 0].T)  # [64, S]
    sin_h = np.ascontiguousarray(freqs_cis[:, :, 1].T)
    cosT = np.concatenate([cos_h, cos_h], axis=0).astype(bf)   # [128, S]
    sinT = np.concatenate([-sin_h, sin_h], axis=0).astype(bf)

    in_maps = []
    for c in range(N_CORES):
        b, g = c // KVH, c % KVH
        hq = slice(NH * g, NH * (g + 1))
        in_maps.append({
            "xT": np.ascontiguousarray(x[b].T).astype(bf),
            "wqT": np.ascontiguousarray(
                wq_p[hq].reshape(NH * HD, DIM).T).astype(bf),
            "wkT": np.ascontiguousarray(wk_p[g].T).astype(bf),
            "wvT": np.ascontiguousarray(wv[g * HD:(g + 1) * HD].T).astype(bf),
            "woT": np.ascontiguousarray(
                wo[:, NH * HD * g:NH * HD * (g + 1)].T).astype(bf),
            "cosT": cosT,
            "sinT": sinT,
        })
    return in_maps


def _reduce_outputs(results):
    out = np.zeros((B, S, DIM), np.float32)
    for c in range(N_CORES):
        out[c // KVH] += results[c]["out"].astype(np.float32)
    return out


def kernel(x, freqs_cis, wq, wk, wv, wo, _trace=False, _trace_kwargs=None):
    nc = _get_nc()
    x, freqs_cis, wq, wk, wv, wo = (
        np.asarray(a, np.float32) for a in (x, freqs_cis, wq, wk, wv, wo))
    in_maps = _prep_in_maps(x, freqs_cis, wq, wk, wv, wo)
    res = run_bass_kernel_spmd(nc, in_maps, core_ids=list(range(N_CORES)),
                               trace=_trace, **(_trace_kwargs or {}))
    out = _reduce_outputs(res.results)
    if _trace:
        _cached["last_exec_time_ns"] = res.exec_time_ns
        _cached["last_results"] = res
    return out
